# revision 21
# baseline (speedup 1.0000x reference)
"""Multi-head attention (B=4, N=2048, C=1024, H=16) on 8 TRN2 NeuronCores.

Tensor-parallel over heads: core c owns heads (2c, 2c+1). Each core computes
q/k/v projections for its heads over all tokens, full attention for its heads,
and its heads' slice of the output projection, producing a [B*N, C] f32
partial; the host sums the 8 partials and adds the projection bias.

Layout strategy (all compute in bf16, f32 accumulation):
  - x is passed transposed ([C, B*N] bf16) so QKV matmuls contract over C
    with no on-device transposes.
  - q, k are produced head-dim-major ([2*64, tok]); scores are computed
    TRANSPOSED (s on partitions, q on free dim) so exp comes straight out of
    PSUM; the two heads' K=64 score matmuls are row-packed into one
    [128, 1024] PSUM pair tile.
  - v is produced token-major with a shared ones column ([v_h0 | 1 | v_h1]);
    the M=65 AV matmul then yields both o^T (rows 0-63) and the softmax
    denominator Z (row 64) in one accumulation group.
  - 1/Z scaling uses a DRAM bounce + partition-broadcast DMA load.
  - The output projection consumes o^T directly as lhsT.
"""
import numpy as np
import ml_dtypes

import concourse.bass as bass
import concourse.mybir as mybir
import concourse.tile as tile
from concourse import bacc
from concourse.bass_utils import run_bass_kernel_spmd

B, N, C, H = 4, 2048, 1024, 16
D = C // H  # 64
NCORES = 8
HPC = H // NCORES  # 2 heads per core

BF16 = mybir.dt.bfloat16
F32 = mybir.dt.float32
nbf16 = ml_dtypes.bfloat16


def build(n_batch=B, tok=N):
    """Emit the per-core program. Parameterized so a small config can be
    simulated; the full size is n_batch=4, tok=2048."""
    t_all = n_batch * tok
    n_c = C // 128            # 8 contraction chunks
    n_qk = (tok + 511) // 512  # qk projection N-chunks
    qk_w = min(512, tok)
    n_tt = tok // 128          # token tiles (v, proj)
    n_s = tok // 128           # kv tiles
    n_qc = (tok + 511) // 512  # attention q chunks
    qc_w = min(512, tok)
    scale = float(D) ** -0.5

    nc = bacc.Bacc("TRN2", target_bir_lowering=False, debug=False,
                   num_devices=NCORES)
    xT_d = nc.dram_tensor("xT", [C, t_all], BF16, kind="ExternalInput")
    wqkT_d = nc.dram_tensor("wqkT", [C, 2 * HPC * D], BF16, kind="ExternalInput")
    wvT_d = nc.dram_tensor("wvT", [C, HPC * D], BF16, kind="ExternalInput")
    wpT_d = nc.dram_tensor("wpT", [HPC * D, C], BF16, kind="ExternalInput")
    out_d = nc.dram_tensor("out", [t_all, C], BF16, kind="ExternalOutput")
    zout_d = nc.dram_tensor("zout", [n_batch * HPC, tok], F32, kind="ExternalOutput")

    with tile.TileContext(nc) as tc:
        with (
            tc.tile_pool(name="singles", bufs=1) as singles,
            tc.tile_pool(name="xt", bufs=10) as xt_p,
            tc.tile_pool(name="qk", bufs=4) as qk_p,
            tc.tile_pool(name="vp", bufs=34) as v_p,
            tc.tile_pool(name="et", bufs=12) as et_p,
            tc.tile_pool(name="oz", bufs=4) as oz_p,
            tc.tile_pool(name="zb", bufs=2) as zb_p,
            tc.tile_pool(name="osc", bufs=2) as os_p,
            tc.tile_pool(name="ot", bufs=2) as ot_p,
            tc.tile_pool(name="po", bufs=3) as po_p,
            tc.tile_pool(name="ps_mm", bufs=2, space="PSUM") as ps_mm,
            tc.tile_pool(name="ps_sc", bufs=2, space="PSUM") as ps_sc,
            tc.tile_pool(name="ps_av", bufs=2, space="PSUM") as ps_av,
        ):
            wqk_sb = singles.tile([128, n_c, 2 * HPC * D], BF16)
            nc.sync.dma_start(wqk_sb[:], wqkT_d.rearrange("(a p) m -> p a m", p=128))
            wv_sb = singles.tile([128, n_c, HPC * D], BF16)
            nc.sync.dma_start(wv_sb[:], wvT_d.rearrange("(a p) m -> p a m", p=128))
            wp_sb = singles.tile([128, C], BF16)
            nc.sync.dma_start(wp_sb[:], wpT_d[:])

            def make_qkv_quanta(b, st):
                """Work quanta (thunks) for batch b's x loads + QKV matmuls."""
                t0 = b * tok
                quanta = []

                def load_x():
                    xts = []
                    for c in range(n_c):
                        xt = xt_p.tile([128, tok], BF16, tag="xt", name=f"xt{b}_{c}")
                        nc.sync.dma_start(
                            xt[:], xT_d[c * 128:(c + 1) * 128, t0:t0 + tok])
                        xts.append(xt)
                    st["xts"] = xts
                quanta.append(load_x)

                st["qkT"] = [None, None]

                def qk_group(m, n0, nn):
                    # pair of n-chunks per group: each lhsT (c-chunk) is loaded
                    # once and streams both chunks, halving LDWEIGHTS count
                    def f():
                        if st["qkT"][m] is None:
                            st["qkT"][m] = qk_p.tile([128, tok], BF16,
                                                     tag="qk", name=f"qk{b}_{m}")
                        dst = st["qkT"][m]
                        pmms = [ps_mm.tile([128, qk_w], F32, tag="mm", name="pmm")
                                for _ in range(nn)]
                        for c in range(n_c):
                            for j in range(nn):
                                n = n0 + j
                                nc.tensor.matmul(
                                    pmms[j][:],
                                    wqk_sb[:, c, m * 128:(m + 1) * 128],
                                    st["xts"][c][:, n * qk_w:(n + 1) * qk_w],
                                    start=(c == 0), stop=(c == n_c - 1),
                                )
                        for j in range(nn):
                            n = n0 + j
                            nc.vector.tensor_copy(
                                dst[:, n * qk_w:(n + 1) * qk_w], pmms[j][:])
                    return f
                for m in range(2):
                    for n0 in range(0, n_qk, 2):
                        quanta.append(qk_group(m, n0, min(2, n_qk - n0)))

                st["vts"] = [None] * n_tt

                def v_group(tt):
                    def f():
                        pv = ps_mm.tile([128, HPC * D], F32, tag="mm", name="pv")
                        for c in range(n_c):
                            nc.tensor.matmul(
                                pv[:],
                                st["xts"][c][:, tt * 128:(tt + 1) * 128],
                                wv_sb[:, c, :],
                                start=(c == 0), stop=(c == n_c - 1),
                            )
                        vt = v_p.tile([128, 2 * D + 2], BF16, tag="vt",
                                      name=f"vt{b}_{tt}")
                        nc.vector.tensor_copy(vt[:, 0:D], pv[:, 0:D])
                        nc.vector.tensor_copy(vt[:, D + 1:2 * D + 1], pv[:, D:2 * D])
                        nc.gpsimd.memset(vt[:, D:D + 1], 1.0)
                        nc.gpsimd.memset(vt[:, 2 * D + 1:2 * D + 2], 1.0)
                        st["vts"][tt] = vt
                    return f
                for tt in range(n_tt):
                    quanta.append(v_group(tt))
                return quanta

            def zproj_qc_quanta(b, st, qc):
                """Per-q-chunk 1/Z scaling + projection for the tt tiles whose
                tokens are fully covered by chunks <= qc."""
                t0 = b * tok
                quanta = []

                def zchain(h):
                    def f():
                        if "oTbig" not in st:
                            st["oTbig"] = ot_p.tile([128, tok], BF16,
                                                    tag="ot", name=f"ot{b}")
                        ozf = st["ozf"]
                        c_sl = slice(qc * qc_w, (qc + 1) * qc_w)
                        zrow = b * HPC + h
                        nc.sync.dma_start(zout_d[zrow:zrow + 1, c_sl],
                                          ozf[h][D:D + 1, c_sl])
                        zb = zb_p.tile([D, qc_w], F32, tag="zb", name="zb")
                        nc.sync.dma_start(
                            zb[:],
                            zout_d[zrow:zrow + 1, c_sl].to_broadcast((D, qc_w)))
                        nc.vector.reciprocal_approx_fast(zb[:], zb[:])
                        ost = os_p.tile([D, qc_w], BF16, tag="ost", name="ost")
                        nc.vector.tensor_mul(ost[:], ozf[h][0:D, c_sl], zb[:])
                        nc.sync.dma_start(st["oTbig"][h * D:(h + 1) * D, c_sl],
                                          ost[:])
                    return f

                def proj_tt(tt):
                    def f():
                        po = po_p.tile([128, C], BF16, tag="po", name="po")
                        for nn in range(C // 512):
                            pp = ps_mm.tile([128, 512], F32, tag="mm", name="pp")
                            nc.tensor.matmul(
                                pp[:],
                                st["oTbig"][:, tt * 128:(tt + 1) * 128],
                                wp_sb[:, nn * 512:(nn + 1) * 512],
                                start=True, stop=True,
                            )
                            nc.vector.tensor_copy(po[:, nn * 512:(nn + 1) * 512],
                                                  pp[:])
                        r0 = t0 + tt * 128
                        nc.sync.dma_start(out_d[r0:r0 + 128, :], po[:])
                    return f

                quanta.append(zchain(0))
                quanta.append(zchain(1))
                for tt in range(qc * qc_w // 128, (qc + 1) * qc_w // 128):
                    quanta.append(proj_tt(tt))
                return quanta

            def phase_attn(b, st, fills, self_push):
                """Transposed scores -> exp -> M=65 AV. Pops fill quanta
                between s-iterations so PE has work while ACT runs the exps;
                pushes this batch's own per-qc Z+projection quanta into the
                stream as each q-chunk's AV completes."""
                qT, kT = st["qkT"]
                vts = st["vts"]
                it = 0
                ozf = [oz_p.tile([D + 1, tok], F32, tag="ozf", name=f"ozf{b}_{h}")
                       for h in range(HPC)]
                st["ozf"] = ozf
                for qc in range(n_qc):
                    q_sl = slice(qc * qc_w, (qc + 1) * qc_w)
                    ets = []
                    pavs = [ps_av.tile([D + 1, qc_w], F32, tag="av", name=f"pav{h}")
                            for h in range(HPC)]

                    def emit_sc(s):
                        psc = ps_sc.tile([128, 2 * qc_w], F32, tag="psc", name="psc")
                        for h in range(HPC):
                            nc.tensor.matmul(
                                psc[:, h * qc_w:(h + 1) * qc_w],
                                kT[h * D:(h + 1) * D, s * 128:(s + 1) * 128],
                                qT[h * D:(h + 1) * D, q_sl],
                                start=True, stop=True,
                            )
                        et = et_p.tile([128, 2 * qc_w], BF16, tag="et", name="et")
                        nc.scalar.activation(et[:], psc[:],
                                             mybir.ActivationFunctionType.Exp,
                                             scale=scale)
                        ets.append(et)

                    def emit_av(s):
                        for h in range(HPC):
                            nc.tensor.matmul(
                                pavs[h][:],
                                vts[s][:, h * (D + 1):(h + 1) * (D + 1)],
                                ets[s][:, h * qc_w:(h + 1) * qc_w],
                                start=(s == 0), stop=(s == n_s - 1),
                            )

                    for s in range(n_s):
                        emit_sc(s)
                        if s >= 2:
                            emit_av(s - 2)
                        if fills and not (b > 0 and it % 4 == 3):
                            fills.pop(0)()
                        it += 1
                    for s in range(max(0, n_s - 2), n_s):
                        emit_av(s)
                    for h in range(HPC):
                        nc.vector.tensor_copy(ozf[h][:, q_sl], pavs[h][:])
                    if self_push:
                        fills.extend(zproj_qc_quanta(b, st, qc))
                # drain any leftover fill quanta
                while fills:
                    fills.pop(0)()

            def interleave2(a, bq):
                out = []
                ia = ib = 0
                while ia < len(a) or ib < len(bq):
                    for _ in range(2):
                        if ia < len(a):
                            out.append(a[ia]); ia += 1
                    if ib < len(bq):
                        out.append(bq[ib]); ib += 1
                return out

            states = [dict() for _ in range(n_batch)]
            q0 = make_qkv_quanta(0, states[0])
            # batch 0: x loads + qk groups up front; v groups become fills
            n_up = 1 + 2 * n_qk
            for q in q0[:n_up]:
                q()
            carry = q0[n_up:]
            prj = []
            for b in range(n_batch):
                last = b == n_batch - 1
                fills = list(carry)
                carry = []
                nxt = make_qkv_quanta(b + 1, states[b + 1]) if not last else []
                if nxt:
                    fills.append(nxt.pop(0))  # x loads first
                fills += prj[:2]              # first z chains early
                fills += interleave2(nxt, prj[2:])
                phase_attn(b, states[b], fills, self_push=last)
                if not last:
                    prj = []
                    for qc in range(n_qc):
                        prj += zproj_qc_quanta(b, states[b], qc)

    nc.compile()
    return nc


def prep_in_maps(x, W_qkv, W_proj, n_batch=B, tok=N):
    """Shard + lay out inputs per core (bf16, transposed as the kernel wants)."""
    t_all = n_batch * tok
    x2 = np.ascontiguousarray(
        np.asarray(x, dtype=np.float32).reshape(t_all, C).T).astype(nbf16)
    Wq = np.asarray(W_qkv[0:C], dtype=np.float32)
    Wk = np.asarray(W_qkv[C:2 * C], dtype=np.float32)
    Wv = np.asarray(W_qkv[2 * C:3 * C], dtype=np.float32)
    Wp = np.asarray(W_proj, dtype=np.float32)
    in_maps = []
    for cid in range(NCORES):
        h0, h1 = HPC * cid, HPC * cid + 1
        r0, r1 = slice(h0 * D, (h0 + 1) * D), slice(h1 * D, (h1 + 1) * D)
        wqk = np.concatenate([Wq[r0], Wq[r1], Wk[r0], Wk[r1]], axis=0)
        wv = np.concatenate([Wv[r0], Wv[r1]], axis=0)
        wp = np.concatenate([Wp[:, r0], Wp[:, r1]], axis=1)
        in_maps.append({
            "xT": x2,
            "wqkT": np.ascontiguousarray(wqk.T).astype(nbf16),
            "wvT": np.ascontiguousarray(wv.T).astype(nbf16),
            "wpT": np.ascontiguousarray(wp.T).astype(nbf16),
        })
    return in_maps


_CACHE = {}


def run(x, W_qkv, W_proj, b_proj, trace=False, trace_kwargs=None):
    key = "full"
    if key not in _CACHE:
        _CACHE[key] = build()
    nc = _CACHE[key]
    in_maps = prep_in_maps(x, W_qkv, W_proj)
    res = run_bass_kernel_spmd(
        nc, in_maps, core_ids=list(range(NCORES)), trace=trace,
        **(trace_kwargs or {}))
    acc = res.results[0]["out"].astype(np.float32)
    for i in range(1, NCORES):
        acc += res.results[i]["out"]
    acc += np.asarray(b_proj, dtype=np.float32)[None, :]
    return acc.reshape(B, N, C), res


def kernel(x, W_qkv, W_proj, b_proj):
    out, _ = run(x, W_qkv, W_proj, b_proj)
    return out


# revision 22
# speedup vs baseline: 1.0080x; 1.0080x over previous
"""Multi-head attention (B=4, N=2048, C=1024, H=16) on 8 TRN2 NeuronCores.

Tensor-parallel over heads: core c owns heads (2c, 2c+1). Each core computes
q/k/v projections for its heads over all tokens, full attention for its heads,
and its heads' slice of the output projection, producing a [B*N, C] f32
partial; the host sums the 8 partials and adds the projection bias.

Layout strategy (all compute in bf16, f32 accumulation):
  - x is passed transposed ([C, B*N] bf16) so QKV matmuls contract over C
    with no on-device transposes.
  - q, k are produced head-dim-major ([2*64, tok]); scores are computed
    TRANSPOSED (s on partitions, q on free dim) so exp comes straight out of
    PSUM; the two heads' K=64 score matmuls are row-packed into one
    [128, 1024] PSUM pair tile.
  - v is produced token-major with a shared ones column ([v_h0 | 1 | v_h1]);
    the M=65 AV matmul then yields both o^T (rows 0-63) and the softmax
    denominator Z (row 64) in one accumulation group.
  - 1/Z scaling uses a DRAM bounce + partition-broadcast DMA load.
  - The output projection consumes o^T directly as lhsT.
"""
import numpy as np
import ml_dtypes

import concourse.bass as bass
import concourse.mybir as mybir
import concourse.tile as tile
from concourse import bacc
from concourse.bass_utils import run_bass_kernel_spmd

B, N, C, H = 4, 2048, 1024, 16
D = C // H  # 64
NCORES = 8
HPC = H // NCORES  # 2 heads per core

BF16 = mybir.dt.bfloat16
F32 = mybir.dt.float32
nbf16 = ml_dtypes.bfloat16


def build(n_batch=B, tok=N):
    """Emit the per-core program. Parameterized so a small config can be
    simulated; the full size is n_batch=4, tok=2048."""
    t_all = n_batch * tok
    n_c = C // 128            # 8 contraction chunks
    n_qk = (tok + 511) // 512  # qk projection N-chunks
    qk_w = min(512, tok)
    n_tt = tok // 128          # token tiles (v, proj)
    n_s = tok // 128           # kv tiles
    n_qc = (tok + 511) // 512  # attention q chunks
    qc_w = min(512, tok)
    scale = float(D) ** -0.5

    nc = bacc.Bacc("TRN2", target_bir_lowering=False, debug=False,
                   num_devices=NCORES)
    xT_d = nc.dram_tensor("xT", [C, t_all], BF16, kind="ExternalInput")
    wqkT_d = nc.dram_tensor("wqkT", [C, 2 * HPC * D], BF16, kind="ExternalInput")
    wvT_d = nc.dram_tensor("wvT", [C, HPC * D], BF16, kind="ExternalInput")
    wpT_d = nc.dram_tensor("wpT", [HPC * D, C], BF16, kind="ExternalInput")
    out_d = nc.dram_tensor("out", [t_all, C], BF16, kind="ExternalOutput")
    zout_d = nc.dram_tensor("zout", [n_batch * HPC, tok], F32, kind="ExternalOutput")

    with tile.TileContext(nc) as tc:
        with (
            tc.tile_pool(name="singles", bufs=1) as singles,
            tc.tile_pool(name="xt", bufs=10) as xt_p,
            tc.tile_pool(name="qk", bufs=4) as qk_p,
            tc.tile_pool(name="vp", bufs=34) as v_p,
            tc.tile_pool(name="et", bufs=12) as et_p,
            tc.tile_pool(name="oz", bufs=4) as oz_p,
            tc.tile_pool(name="zb", bufs=2) as zb_p,
            tc.tile_pool(name="osc", bufs=2) as os_p,
            tc.tile_pool(name="ot", bufs=2) as ot_p,
            tc.tile_pool(name="po", bufs=3) as po_p,
            tc.tile_pool(name="ps_mm", bufs=2, space="PSUM") as ps_mm,
            tc.tile_pool(name="ps_sc", bufs=2, space="PSUM") as ps_sc,
            tc.tile_pool(name="ps_av", bufs=2, space="PSUM") as ps_av,
        ):
            wqk_sb = singles.tile([128, n_c, 2 * HPC * D], BF16)
            nc.sync.dma_start(wqk_sb[:], wqkT_d.rearrange("(a p) m -> p a m", p=128))
            wv_sb = singles.tile([128, n_c, HPC * D], BF16)
            nc.sync.dma_start(wv_sb[:], wvT_d.rearrange("(a p) m -> p a m", p=128))
            wp_sb = singles.tile([128, C], BF16)
            nc.sync.dma_start(wp_sb[:], wpT_d[:])

            def make_qkv_quanta(b, st):
                """Work quanta (thunks) for batch b's x loads + QKV matmuls."""
                t0 = b * tok
                quanta = []

                def load_x():
                    xts = []
                    for c in range(n_c):
                        xt = xt_p.tile([128, tok], BF16, tag="xt", name=f"xt{b}_{c}")
                        nc.sync.dma_start(
                            xt[:], xT_d[c * 128:(c + 1) * 128, t0:t0 + tok])
                        xts.append(xt)
                    st["xts"] = xts
                quanta.append(load_x)

                st["qkT"] = [None, None]

                def qk_group(m, n):
                    def f():
                        if st["qkT"][m] is None:
                            st["qkT"][m] = qk_p.tile([128, tok], BF16,
                                                     tag="qk", name=f"qk{b}_{m}")
                        dst = st["qkT"][m]
                        pmm = ps_mm.tile([128, qk_w], F32, tag="mm", name="pmm")
                        for c in range(n_c):
                            nc.tensor.matmul(
                                pmm[:],
                                wqk_sb[:, c, m * 128:(m + 1) * 128],
                                st["xts"][c][:, n * qk_w:(n + 1) * qk_w],
                                start=(c == 0), stop=(c == n_c - 1),
                            )
                        nc.vector.tensor_copy(dst[:, n * qk_w:(n + 1) * qk_w], pmm[:])
                    return f
                for m in range(2):
                    for n in range(n_qk):
                        quanta.append(qk_group(m, n))

                st["vts"] = [None] * n_tt

                def v_group(tt):
                    def f():
                        pv = ps_mm.tile([128, HPC * D], F32, tag="mm", name="pv")
                        for c in range(n_c):
                            nc.tensor.matmul(
                                pv[:],
                                st["xts"][c][:, tt * 128:(tt + 1) * 128],
                                wv_sb[:, c, :],
                                start=(c == 0), stop=(c == n_c - 1),
                            )
                        vt = v_p.tile([128, 2 * D + 2], BF16, tag="vt",
                                      name=f"vt{b}_{tt}")
                        nc.vector.tensor_copy(vt[:, 0:D], pv[:, 0:D])
                        nc.vector.tensor_copy(vt[:, D + 1:2 * D + 1], pv[:, D:2 * D])
                        nc.gpsimd.memset(vt[:, D:D + 1], 1.0)
                        nc.gpsimd.memset(vt[:, 2 * D + 1:2 * D + 2], 1.0)
                        st["vts"][tt] = vt
                    return f
                for tt in range(n_tt):
                    quanta.append(v_group(tt))
                return quanta

            def zproj_qc_quanta(b, st, qc):
                """Per-q-chunk 1/Z scaling + projection for the tt tiles whose
                tokens are fully covered by chunks <= qc."""
                t0 = b * tok
                quanta = []

                def zchain(h):
                    def f():
                        if "oTbig" not in st:
                            st["oTbig"] = ot_p.tile([128, tok], BF16,
                                                    tag="ot", name=f"ot{b}")
                        ozf = st["ozf"]
                        c_sl = slice(qc * qc_w, (qc + 1) * qc_w)
                        zrow = b * HPC + h
                        nc.sync.dma_start(zout_d[zrow:zrow + 1, c_sl],
                                          ozf[h][D:D + 1, c_sl])
                        zb = zb_p.tile([D, qc_w], F32, tag="zb", name="zb")
                        nc.sync.dma_start(
                            zb[:],
                            zout_d[zrow:zrow + 1, c_sl].to_broadcast((D, qc_w)))
                        nc.vector.reciprocal_approx_fast(zb[:], zb[:])
                        ost = os_p.tile([D, qc_w], BF16, tag="ost", name="ost")
                        nc.vector.tensor_mul(ost[:], ozf[h][0:D, c_sl], zb[:])
                        nc.sync.dma_start(st["oTbig"][h * D:(h + 1) * D, c_sl],
                                          ost[:])
                    return f

                def proj_tt(tt):
                    def f():
                        po = po_p.tile([128, C], BF16, tag="po", name="po")
                        for nn in range(C // 512):
                            pp = ps_mm.tile([128, 512], F32, tag="mm", name="pp")
                            nc.tensor.matmul(
                                pp[:],
                                st["oTbig"][:, tt * 128:(tt + 1) * 128],
                                wp_sb[:, nn * 512:(nn + 1) * 512],
                                start=True, stop=True,
                            )
                            nc.vector.tensor_copy(po[:, nn * 512:(nn + 1) * 512],
                                                  pp[:])
                        r0 = t0 + tt * 128
                        nc.sync.dma_start(out_d[r0:r0 + 128, :], po[:])
                    return f

                quanta.append(zchain(0))
                quanta.append(zchain(1))
                for tt in range(qc * qc_w // 128, (qc + 1) * qc_w // 128):
                    quanta.append(proj_tt(tt))
                return quanta

            def phase_attn(b, st, fills, self_push):
                """Transposed scores -> exp -> M=65 AV. Pops fill quanta
                between s-iterations so PE has work while ACT runs the exps;
                pushes this batch's own per-qc Z+projection quanta into the
                stream as each q-chunk's AV completes."""
                qT, kT = st["qkT"]
                vts = st["vts"]
                it = 0
                ozf = [oz_p.tile([D + 1, tok], F32, tag="ozf", name=f"ozf{b}_{h}")
                       for h in range(HPC)]
                st["ozf"] = ozf
                for qc in range(n_qc):
                    q_sl = slice(qc * qc_w, (qc + 1) * qc_w)
                    ets = []
                    pavs = [ps_av.tile([D + 1, qc_w], F32, tag="av", name=f"pav{h}")
                            for h in range(HPC)]

                    def emit_sc(s):
                        psc = ps_sc.tile([128, 2 * qc_w], F32, tag="psc", name="psc")
                        for h in range(HPC):
                            nc.tensor.matmul(
                                psc[:, h * qc_w:(h + 1) * qc_w],
                                kT[h * D:(h + 1) * D, s * 128:(s + 1) * 128],
                                qT[h * D:(h + 1) * D, q_sl],
                                start=True, stop=True,
                            )
                        et = et_p.tile([128, 2 * qc_w], BF16, tag="et", name="et")
                        nc.scalar.activation(et[:], psc[:],
                                             mybir.ActivationFunctionType.Exp,
                                             scale=scale)
                        ets.append(et)

                    def emit_av(s):
                        for h in range(HPC):
                            nc.tensor.matmul(
                                pavs[h][:],
                                vts[s][:, h * (D + 1):(h + 1) * (D + 1)],
                                ets[s][:, h * qc_w:(h + 1) * qc_w],
                                start=(s == 0), stop=(s == n_s - 1),
                            )

                    for s in range(n_s):
                        emit_sc(s)
                        if s >= 2:
                            emit_av(s - 2)
                        if fills and not (b > 0 and it % 4 == 3):
                            fills.pop(0)()
                        it += 1
                    for s in range(max(0, n_s - 2), n_s):
                        emit_av(s)
                    for h in range(HPC):
                        nc.vector.tensor_copy(ozf[h][:, q_sl], pavs[h][:])
                    if self_push:
                        fills.extend(zproj_qc_quanta(b, st, qc))
                # drain any leftover fill quanta
                while fills:
                    fills.pop(0)()

            def interleave2(a, bq):
                out = []
                ia = ib = 0
                while ia < len(a) or ib < len(bq):
                    for _ in range(2):
                        if ia < len(a):
                            out.append(a[ia]); ia += 1
                    if ib < len(bq):
                        out.append(bq[ib]); ib += 1
                return out

            states = [dict() for _ in range(n_batch)]
            q0 = make_qkv_quanta(0, states[0])
            # batch 0: x loads + qk groups up front; v groups become fills
            n_up = 1 + 2 * n_qk
            for q in q0[:n_up]:
                q()
            carry = q0[n_up:]
            prj = []
            for b in range(n_batch):
                last = b == n_batch - 1
                fills = list(carry)
                carry = []
                nxt = make_qkv_quanta(b + 1, states[b + 1]) if not last else []
                if nxt:
                    fills.append(nxt.pop(0))  # x loads first
                fills += prj[:2]              # first z chains early
                fills += interleave2(nxt, prj[2:])
                phase_attn(b, states[b], fills, self_push=last)
                if not last:
                    prj = []
                    for qc in range(n_qc):
                        prj += zproj_qc_quanta(b, states[b], qc)

    nc.compile()
    return nc


def prep_in_maps(x, W_qkv, W_proj, n_batch=B, tok=N):
    """Shard + lay out inputs per core (bf16, transposed as the kernel wants)."""
    t_all = n_batch * tok
    x2 = np.ascontiguousarray(
        np.asarray(x, dtype=np.float32).reshape(t_all, C).T).astype(nbf16)
    Wq = np.asarray(W_qkv[0:C], dtype=np.float32)
    Wk = np.asarray(W_qkv[C:2 * C], dtype=np.float32)
    Wv = np.asarray(W_qkv[2 * C:3 * C], dtype=np.float32)
    Wp = np.asarray(W_proj, dtype=np.float32)
    in_maps = []
    for cid in range(NCORES):
        h0, h1 = HPC * cid, HPC * cid + 1
        r0, r1 = slice(h0 * D, (h0 + 1) * D), slice(h1 * D, (h1 + 1) * D)
        wqk = np.concatenate([Wq[r0], Wq[r1], Wk[r0], Wk[r1]], axis=0)
        wv = np.concatenate([Wv[r0], Wv[r1]], axis=0)
        wp = np.concatenate([Wp[:, r0], Wp[:, r1]], axis=1)
        in_maps.append({
            "xT": x2,
            "wqkT": np.ascontiguousarray(wqk.T).astype(nbf16),
            "wvT": np.ascontiguousarray(wv.T).astype(nbf16),
            "wpT": np.ascontiguousarray(wp.T).astype(nbf16),
        })
    return in_maps


_CACHE = {}


def run(x, W_qkv, W_proj, b_proj, trace=False, trace_kwargs=None):
    key = "full"
    if key not in _CACHE:
        _CACHE[key] = build()
    nc = _CACHE[key]
    in_maps = prep_in_maps(x, W_qkv, W_proj)
    res = run_bass_kernel_spmd(
        nc, in_maps, core_ids=list(range(NCORES)), trace=trace,
        **(trace_kwargs or {}))
    acc = res.results[0]["out"].astype(np.float32)
    for i in range(1, NCORES):
        acc += res.results[i]["out"]
    acc += np.asarray(b_proj, dtype=np.float32)[None, :]
    return acc.reshape(B, N, C), res


def kernel(x, W_qkv, W_proj, b_proj):
    out, _ = run(x, W_qkv, W_proj, b_proj)
    return out


# revision 23
# speedup vs baseline: 1.0211x; 1.0130x over previous
"""Multi-head attention (B=4, N=2048, C=1024, H=16) on 8 TRN2 NeuronCores.

Tensor-parallel over heads: core c owns heads (2c, 2c+1). Each core computes
q/k/v projections for its heads over all tokens, full attention for its heads,
and its heads' slice of the output projection, producing a [B*N, C] f32
partial; the host sums the 8 partials and adds the projection bias.

Layout strategy (all compute in bf16, f32 accumulation):
  - x is passed transposed ([C, B*N] bf16) so QKV matmuls contract over C
    with no on-device transposes.
  - q, k are produced head-dim-major ([2*64, tok]); scores are computed
    TRANSPOSED (s on partitions, q on free dim) so exp comes straight out of
    PSUM; the two heads' K=64 score matmuls are row-packed into one
    [128, 1024] PSUM pair tile.
  - v is produced token-major with a shared ones column ([v_h0 | 1 | v_h1]);
    the M=65 AV matmul then yields both o^T (rows 0-63) and the softmax
    denominator Z (row 64) in one accumulation group.
  - 1/Z scaling uses a DRAM bounce + partition-broadcast DMA load.
  - The output projection consumes o^T directly as lhsT.
"""
import numpy as np
import ml_dtypes

import concourse.bass as bass
import concourse.mybir as mybir
import concourse.tile as tile
from concourse import bacc
from concourse.bass_utils import run_bass_kernel_spmd

B, N, C, H = 4, 2048, 1024, 16
D = C // H  # 64
NCORES = 8
HPC = H // NCORES  # 2 heads per core

BF16 = mybir.dt.bfloat16
F32 = mybir.dt.float32
nbf16 = ml_dtypes.bfloat16


def build(n_batch=B, tok=N):
    """Emit the per-core program. Parameterized so a small config can be
    simulated; the full size is n_batch=4, tok=2048."""
    t_all = n_batch * tok
    n_c = C // 128            # 8 contraction chunks
    n_qk = (tok + 511) // 512  # qk projection N-chunks
    qk_w = min(512, tok)
    n_tt = tok // 128          # token tiles (v, proj)
    n_s = tok // 128           # kv tiles
    n_qc = (tok + 511) // 512  # attention q chunks
    qc_w = min(512, tok)
    scale = float(D) ** -0.5

    nc = bacc.Bacc("TRN2", target_bir_lowering=False, debug=False,
                   num_devices=NCORES)
    xT_d = nc.dram_tensor("xT", [C, t_all], BF16, kind="ExternalInput")
    wqkT_d = nc.dram_tensor("wqkT", [C, 2 * HPC * D], BF16, kind="ExternalInput")
    wvT_d = nc.dram_tensor("wvT", [C, HPC * D], BF16, kind="ExternalInput")
    wpT_d = nc.dram_tensor("wpT", [HPC * D, C], BF16, kind="ExternalInput")
    out_d = nc.dram_tensor("out", [t_all, C], BF16, kind="ExternalOutput")
    zout_d = nc.dram_tensor("zout", [n_batch * HPC, tok], F32, kind="ExternalOutput")

    with tile.TileContext(nc) as tc:
        with (
            tc.tile_pool(name="singles", bufs=1) as singles,
            tc.tile_pool(name="xt", bufs=10) as xt_p,
            tc.tile_pool(name="qk", bufs=4) as qk_p,
            tc.tile_pool(name="vp", bufs=34) as v_p,
            tc.tile_pool(name="et", bufs=14) as et_p,
            tc.tile_pool(name="oz", bufs=4) as oz_p,
            tc.tile_pool(name="zb", bufs=2) as zb_p,
            tc.tile_pool(name="osc", bufs=2) as os_p,
            tc.tile_pool(name="ot", bufs=2) as ot_p,
            tc.tile_pool(name="po", bufs=4) as po_p,
            tc.tile_pool(name="ps_mm", bufs=2, space="PSUM") as ps_mm,
            tc.tile_pool(name="ps_sc", bufs=2, space="PSUM") as ps_sc,
            tc.tile_pool(name="ps_av", bufs=2, space="PSUM") as ps_av,
        ):
            wqk_sb = singles.tile([128, n_c, 2 * HPC * D], BF16)
            nc.sync.dma_start(wqk_sb[:], wqkT_d.rearrange("(a p) m -> p a m", p=128))
            wv_sb = singles.tile([128, n_c, HPC * D], BF16)
            nc.sync.dma_start(wv_sb[:], wvT_d.rearrange("(a p) m -> p a m", p=128))
            wp_sb = singles.tile([128, C], BF16)
            nc.sync.dma_start(wp_sb[:], wpT_d[:])

            def make_qkv_quanta(b, st):
                """Work quanta (thunks) for batch b's x loads + QKV matmuls."""
                t0 = b * tok
                quanta = []

                def load_x():
                    xts = []
                    for c in range(n_c):
                        xt = xt_p.tile([128, tok], BF16, tag="xt", name=f"xt{b}_{c}")
                        nc.sync.dma_start(
                            xt[:], xT_d[c * 128:(c + 1) * 128, t0:t0 + tok])
                        xts.append(xt)
                    st["xts"] = xts
                quanta.append(load_x)

                st["qkT"] = [None, None]

                def qk_group(m, n):
                    def f():
                        if st["qkT"][m] is None:
                            st["qkT"][m] = qk_p.tile([128, tok], BF16,
                                                     tag="qk", name=f"qk{b}_{m}")
                        dst = st["qkT"][m]
                        pmm = ps_mm.tile([128, qk_w], F32, tag="mm", name="pmm")
                        for c in range(n_c):
                            nc.tensor.matmul(
                                pmm[:],
                                wqk_sb[:, c, m * 128:(m + 1) * 128],
                                st["xts"][c][:, n * qk_w:(n + 1) * qk_w],
                                start=(c == 0), stop=(c == n_c - 1),
                            )
                        nc.vector.tensor_copy(dst[:, n * qk_w:(n + 1) * qk_w], pmm[:])
                    return f
                for m in range(2):
                    for n in range(n_qk):
                        quanta.append(qk_group(m, n))

                st["vts"] = [None] * n_tt

                def v_group(tt):
                    def f():
                        pv = ps_mm.tile([128, HPC * D], F32, tag="mm", name="pv")
                        for c in range(n_c):
                            nc.tensor.matmul(
                                pv[:],
                                st["xts"][c][:, tt * 128:(tt + 1) * 128],
                                wv_sb[:, c, :],
                                start=(c == 0), stop=(c == n_c - 1),
                            )
                        vt = v_p.tile([128, 2 * D + 2], BF16, tag="vt",
                                      name=f"vt{b}_{tt}")
                        nc.vector.tensor_copy(vt[:, 0:D], pv[:, 0:D])
                        nc.vector.tensor_copy(vt[:, D + 1:2 * D + 1], pv[:, D:2 * D])
                        nc.gpsimd.memset(vt[:, D:D + 1], 1.0)
                        nc.gpsimd.memset(vt[:, 2 * D + 1:2 * D + 2], 1.0)
                        st["vts"][tt] = vt
                    return f
                for tt in range(n_tt):
                    quanta.append(v_group(tt))
                return quanta

            def zproj_qc_quanta(b, st, qc):
                """Per-q-chunk 1/Z scaling + projection for the tt tiles whose
                tokens are fully covered by chunks <= qc."""
                t0 = b * tok
                quanta = []

                def zchain(h):
                    def f():
                        if "oTbig" not in st:
                            st["oTbig"] = ot_p.tile([128, tok], BF16,
                                                    tag="ot", name=f"ot{b}")
                        ozf = st["ozf"]
                        c_sl = slice(qc * qc_w, (qc + 1) * qc_w)
                        zrow = b * HPC + h
                        nc.sync.dma_start(zout_d[zrow:zrow + 1, c_sl],
                                          ozf[h][D:D + 1, c_sl])
                        zb = zb_p.tile([D, qc_w], F32, tag="zb", name="zb")
                        nc.sync.dma_start(
                            zb[:],
                            zout_d[zrow:zrow + 1, c_sl].to_broadcast((D, qc_w)))
                        nc.vector.reciprocal_approx_fast(zb[:], zb[:])
                        ost = os_p.tile([D, qc_w], BF16, tag="ost", name="ost")
                        nc.vector.tensor_mul(ost[:], ozf[h][0:D, c_sl], zb[:])
                        nc.sync.dma_start(st["oTbig"][h * D:(h + 1) * D, c_sl],
                                          ost[:])
                    return f

                def proj_tt(tt):
                    def f():
                        po = po_p.tile([128, C], BF16, tag="po", name="po")
                        for nn in range(C // 512):
                            pp = ps_mm.tile([128, 512], F32, tag="mm", name="pp")
                            nc.tensor.matmul(
                                pp[:],
                                st["oTbig"][:, tt * 128:(tt + 1) * 128],
                                wp_sb[:, nn * 512:(nn + 1) * 512],
                                start=True, stop=True,
                            )
                            nc.vector.tensor_copy(po[:, nn * 512:(nn + 1) * 512],
                                                  pp[:])
                        r0 = t0 + tt * 128
                        nc.sync.dma_start(out_d[r0:r0 + 128, :], po[:])
                    return f

                quanta.append(zchain(0))
                quanta.append(zchain(1))
                for tt in range(qc * qc_w // 128, (qc + 1) * qc_w // 128):
                    quanta.append(proj_tt(tt))
                return quanta

            def phase_attn(b, st, fills, self_push):
                """Transposed scores -> exp -> M=65 AV. Pops fill quanta
                between s-iterations so PE has work while ACT runs the exps;
                pushes this batch's own per-qc Z+projection quanta into the
                stream as each q-chunk's AV completes."""
                qT, kT = st["qkT"]
                vts = st["vts"]
                it = 0
                ozf = [oz_p.tile([D + 1, tok], F32, tag="ozf", name=f"ozf{b}_{h}")
                       for h in range(HPC)]
                st["ozf"] = ozf
                for qc in range(n_qc):
                    q_sl = slice(qc * qc_w, (qc + 1) * qc_w)
                    ets = []
                    pavs = [ps_av.tile([D + 1, qc_w], F32, tag="av", name=f"pav{h}")
                            for h in range(HPC)]

                    def emit_sc(s):
                        psc = ps_sc.tile([128, 2 * qc_w], F32, tag="psc", name="psc")
                        for h in range(HPC):
                            nc.tensor.matmul(
                                psc[:, h * qc_w:(h + 1) * qc_w],
                                kT[h * D:(h + 1) * D, s * 128:(s + 1) * 128],
                                qT[h * D:(h + 1) * D, q_sl],
                                start=True, stop=True,
                            )
                        et = et_p.tile([128, 2 * qc_w], BF16, tag="et", name="et")
                        nc.scalar.activation(et[:], psc[:],
                                             mybir.ActivationFunctionType.Exp,
                                             scale=scale)
                        ets.append(et)

                    def emit_av(s):
                        for h in range(HPC):
                            nc.tensor.matmul(
                                pavs[h][:],
                                vts[s][:, h * (D + 1):(h + 1) * (D + 1)],
                                ets[s][:, h * qc_w:(h + 1) * qc_w],
                                start=(s == 0), stop=(s == n_s - 1),
                            )

                    for s in range(n_s):
                        emit_sc(s)
                        if s >= 2:
                            emit_av(s - 2)
                        if fills and not (b > 0 and it % 4 == 3):
                            fills.pop(0)()
                        it += 1
                    for s in range(max(0, n_s - 2), n_s):
                        emit_av(s)
                    for h in range(HPC):
                        nc.vector.tensor_copy(ozf[h][:, q_sl], pavs[h][:])
                    if self_push:
                        fills.extend(zproj_qc_quanta(b, st, qc))
                # drain any leftover fill quanta
                while fills:
                    fills.pop(0)()

            def interleave2(a, bq):
                out = []
                ia = ib = 0
                while ia < len(a) or ib < len(bq):
                    for _ in range(2):
                        if ia < len(a):
                            out.append(a[ia]); ia += 1
                    if ib < len(bq):
                        out.append(bq[ib]); ib += 1
                return out

            states = [dict() for _ in range(n_batch)]
            q0 = make_qkv_quanta(0, states[0])
            # batch 0: x loads + qk groups up front; v groups become fills
            n_up = 1 + 2 * n_qk
            for q in q0[:n_up]:
                q()
            carry = q0[n_up:]
            prj = []
            for b in range(n_batch):
                last = b == n_batch - 1
                fills = list(carry)
                carry = []
                nxt = make_qkv_quanta(b + 1, states[b + 1]) if not last else []
                if nxt:
                    fills.append(nxt.pop(0))  # x loads first
                fills += prj[:2]              # first z chains early
                fills += interleave2(nxt, prj[2:])
                phase_attn(b, states[b], fills, self_push=last)
                if not last:
                    prj = []
                    for qc in range(n_qc):
                        prj += zproj_qc_quanta(b, states[b], qc)

    nc.compile()
    return nc


def prep_in_maps(x, W_qkv, W_proj, n_batch=B, tok=N):
    """Shard + lay out inputs per core (bf16, transposed as the kernel wants)."""
    t_all = n_batch * tok
    x2 = np.ascontiguousarray(
        np.asarray(x, dtype=np.float32).reshape(t_all, C).T).astype(nbf16)
    Wq = np.asarray(W_qkv[0:C], dtype=np.float32)
    Wk = np.asarray(W_qkv[C:2 * C], dtype=np.float32)
    Wv = np.asarray(W_qkv[2 * C:3 * C], dtype=np.float32)
    Wp = np.asarray(W_proj, dtype=np.float32)
    in_maps = []
    for cid in range(NCORES):
        h0, h1 = HPC * cid, HPC * cid + 1
        r0, r1 = slice(h0 * D, (h0 + 1) * D), slice(h1 * D, (h1 + 1) * D)
        wqk = np.concatenate([Wq[r0], Wq[r1], Wk[r0], Wk[r1]], axis=0)
        wv = np.concatenate([Wv[r0], Wv[r1]], axis=0)
        wp = np.concatenate([Wp[:, r0], Wp[:, r1]], axis=1)
        in_maps.append({
            "xT": x2,
            "wqkT": np.ascontiguousarray(wqk.T).astype(nbf16),
            "wvT": np.ascontiguousarray(wv.T).astype(nbf16),
            "wpT": np.ascontiguousarray(wp.T).astype(nbf16),
        })
    return in_maps


_CACHE = {}


def run(x, W_qkv, W_proj, b_proj, trace=False, trace_kwargs=None):
    key = "full"
    if key not in _CACHE:
        _CACHE[key] = build()
    nc = _CACHE[key]
    in_maps = prep_in_maps(x, W_qkv, W_proj)
    res = run_bass_kernel_spmd(
        nc, in_maps, core_ids=list(range(NCORES)), trace=trace,
        **(trace_kwargs or {}))
    acc = res.results[0]["out"].astype(np.float32)
    for i in range(1, NCORES):
        acc += res.results[i]["out"]
    acc += np.asarray(b_proj, dtype=np.float32)[None, :]
    return acc.reshape(B, N, C), res


def kernel(x, W_qkv, W_proj, b_proj):
    out, _ = run(x, W_qkv, W_proj, b_proj)
    return out


# revision 24
# speedup vs baseline: 1.0219x; 1.0007x over previous
"""Multi-head attention (B=4, N=2048, C=1024, H=16) on 8 TRN2 NeuronCores.

Tensor-parallel over heads: core c owns heads (2c, 2c+1). Each core computes
q/k/v projections for its heads over all tokens, full attention for its heads,
and its heads' slice of the output projection, producing a [B*N, C] f32
partial; the host sums the 8 partials and adds the projection bias.

Layout strategy (all compute in bf16, f32 accumulation):
  - x is passed transposed ([C, B*N] bf16) so QKV matmuls contract over C
    with no on-device transposes.
  - q, k are produced head-dim-major ([2*64, tok]); scores are computed
    TRANSPOSED (s on partitions, q on free dim) so exp comes straight out of
    PSUM; the two heads' K=64 score matmuls are row-packed into one
    [128, 1024] PSUM pair tile.
  - v is produced token-major with a shared ones column ([v_h0 | 1 | v_h1]);
    the M=65 AV matmul then yields both o^T (rows 0-63) and the softmax
    denominator Z (row 64) in one accumulation group.
  - 1/Z scaling uses a DRAM bounce + partition-broadcast DMA load.
  - The output projection consumes o^T directly as lhsT.
"""
import numpy as np
import ml_dtypes

import concourse.bass as bass
import concourse.mybir as mybir
import concourse.tile as tile
from concourse import bacc
from concourse.bass_utils import run_bass_kernel_spmd

B, N, C, H = 4, 2048, 1024, 16
D = C // H  # 64
NCORES = 8
HPC = H // NCORES  # 2 heads per core

BF16 = mybir.dt.bfloat16
F32 = mybir.dt.float32
nbf16 = ml_dtypes.bfloat16


def build(n_batch=B, tok=N):
    """Emit the per-core program. Parameterized so a small config can be
    simulated; the full size is n_batch=4, tok=2048."""
    t_all = n_batch * tok
    n_c = C // 128            # 8 contraction chunks
    n_qk = (tok + 511) // 512  # qk projection N-chunks
    qk_w = min(512, tok)
    n_tt = tok // 128          # token tiles (v, proj)
    n_s = tok // 128           # kv tiles
    n_qc = (tok + 511) // 512  # attention q chunks
    qc_w = min(512, tok)
    scale = float(D) ** -0.5

    nc = bacc.Bacc("TRN2", target_bir_lowering=False, debug=False,
                   num_devices=NCORES)
    xT_d = nc.dram_tensor("xT", [C, t_all], BF16, kind="ExternalInput")
    wqkT_d = nc.dram_tensor("wqkT", [C, 2 * HPC * D], BF16, kind="ExternalInput")
    wvT_d = nc.dram_tensor("wvT", [C, HPC * D], BF16, kind="ExternalInput")
    wpT_d = nc.dram_tensor("wpT", [HPC * D, C], BF16, kind="ExternalInput")
    out_d = nc.dram_tensor("out", [t_all, C], BF16, kind="ExternalOutput")
    zout_d = nc.dram_tensor("zout", [n_batch * HPC, tok], F32, kind="ExternalOutput")

    with tile.TileContext(nc) as tc:
        with (
            tc.tile_pool(name="singles", bufs=1) as singles,
            tc.tile_pool(name="xt", bufs=10) as xt_p,
            tc.tile_pool(name="qk", bufs=4) as qk_p,
            tc.tile_pool(name="vp", bufs=34) as v_p,
            tc.tile_pool(name="et", bufs=14) as et_p,
            tc.tile_pool(name="oz", bufs=4) as oz_p,
            tc.tile_pool(name="zb", bufs=2) as zb_p,
            tc.tile_pool(name="osc", bufs=2) as os_p,
            tc.tile_pool(name="ot", bufs=2) as ot_p,
            tc.tile_pool(name="po", bufs=4) as po_p,
            tc.tile_pool(name="ps_mm", bufs=2, space="PSUM") as ps_mm,
            tc.tile_pool(name="ps_sc", bufs=2, space="PSUM") as ps_sc,
            tc.tile_pool(name="ps_av", bufs=2, space="PSUM") as ps_av,
        ):
            wqk_sb = singles.tile([128, n_c, 2 * HPC * D], BF16)
            nc.sync.dma_start(wqk_sb[:], wqkT_d.rearrange("(a p) m -> p a m", p=128))
            wv_sb = singles.tile([128, n_c, HPC * D], BF16)
            nc.sync.dma_start(wv_sb[:], wvT_d.rearrange("(a p) m -> p a m", p=128))
            wp_sb = singles.tile([128, C], BF16)
            nc.sync.dma_start(wp_sb[:], wpT_d[:])

            def make_qkv_quanta(b, st):
                """Work quanta (thunks) for batch b's x loads + QKV matmuls."""
                t0 = b * tok
                quanta = []

                def load_x():
                    xts = []
                    for c in range(n_c):
                        xt = xt_p.tile([128, tok], BF16, tag="xt", name=f"xt{b}_{c}")
                        nc.sync.dma_start(
                            xt[:], xT_d[c * 128:(c + 1) * 128, t0:t0 + tok])
                        xts.append(xt)
                    st["xts"] = xts
                quanta.append(load_x)

                st["qkT"] = [None, None]

                def qk_group(m, n):
                    def f():
                        if st["qkT"][m] is None:
                            st["qkT"][m] = qk_p.tile([128, tok], BF16,
                                                     tag="qk", name=f"qk{b}_{m}")
                        dst = st["qkT"][m]
                        pmm = ps_mm.tile([128, qk_w], F32, tag="mm", name="pmm")
                        for c in range(n_c):
                            nc.tensor.matmul(
                                pmm[:],
                                wqk_sb[:, c, m * 128:(m + 1) * 128],
                                st["xts"][c][:, n * qk_w:(n + 1) * qk_w],
                                start=(c == 0), stop=(c == n_c - 1),
                            )
                        nc.vector.tensor_copy(dst[:, n * qk_w:(n + 1) * qk_w], pmm[:])
                    return f
                for m in range(2):
                    for n in range(n_qk):
                        quanta.append(qk_group(m, n))

                st["vts"] = [None] * n_tt

                def v_group(tt):
                    def f():
                        pv = ps_mm.tile([128, HPC * D], F32, tag="mm", name="pv")
                        for c in range(n_c):
                            nc.tensor.matmul(
                                pv[:],
                                st["xts"][c][:, tt * 128:(tt + 1) * 128],
                                wv_sb[:, c, :],
                                start=(c == 0), stop=(c == n_c - 1),
                            )
                        vt = v_p.tile([128, 2 * D + 2], BF16, tag="vt",
                                      name=f"vt{b}_{tt}")
                        nc.vector.tensor_copy(vt[:, 0:D], pv[:, 0:D])
                        nc.vector.tensor_copy(vt[:, D + 1:2 * D + 1], pv[:, D:2 * D])
                        nc.gpsimd.memset(vt[:, D:D + 1], 1.0)
                        nc.gpsimd.memset(vt[:, 2 * D + 1:2 * D + 2], 1.0)
                        st["vts"][tt] = vt
                    return f
                for tt in range(n_tt):
                    quanta.append(v_group(tt))
                return quanta

            def zproj_qc_quanta(b, st, qc):
                """Per-q-chunk 1/Z scaling + projection for the tt tiles whose
                tokens are fully covered by chunks <= qc."""
                t0 = b * tok
                quanta = []

                def zchain(h):
                    def f():
                        if "oTbig" not in st:
                            st["oTbig"] = ot_p.tile([128, tok], BF16,
                                                    tag="ot", name=f"ot{b}")
                        ozf = st["ozf"]
                        c_sl = slice(qc * qc_w, (qc + 1) * qc_w)
                        zrow = b * HPC + h
                        nc.sync.dma_start(zout_d[zrow:zrow + 1, c_sl],
                                          ozf[h][D:D + 1, c_sl])
                        zb = zb_p.tile([D, qc_w], F32, tag="zb", name="zb")
                        nc.sync.dma_start(
                            zb[:],
                            zout_d[zrow:zrow + 1, c_sl].to_broadcast((D, qc_w)))
                        nc.vector.reciprocal_approx_fast(zb[:], zb[:])
                        ost = os_p.tile([D, qc_w], BF16, tag="ost", name="ost")
                        nc.vector.tensor_mul(ost[:], ozf[h][0:D, c_sl], zb[:])
                        nc.sync.dma_start(st["oTbig"][h * D:(h + 1) * D, c_sl],
                                          ost[:])
                    return f

                def proj_tt(tt):
                    def f():
                        po = po_p.tile([128, C], BF16, tag="po", name="po")
                        for nn in range(C // 512):
                            pp = ps_mm.tile([128, 512], F32, tag="mm", name="pp")
                            nc.tensor.matmul(
                                pp[:],
                                st["oTbig"][:, tt * 128:(tt + 1) * 128],
                                wp_sb[:, nn * 512:(nn + 1) * 512],
                                start=True, stop=True,
                            )
                            nc.vector.tensor_copy(po[:, nn * 512:(nn + 1) * 512],
                                                  pp[:])
                        r0 = t0 + tt * 128
                        nc.sync.dma_start(out_d[r0:r0 + 128, :], po[:])
                    return f

                quanta.append(zchain(0))
                quanta.append(zchain(1))
                for tt in range(qc * qc_w // 128, (qc + 1) * qc_w // 128):
                    quanta.append(proj_tt(tt))
                return quanta

            def phase_attn(b, st, fills, self_push):
                """Transposed scores -> exp -> M=65 AV. Pops fill quanta
                between s-iterations so PE has work while ACT runs the exps;
                pushes this batch's own per-qc Z+projection quanta into the
                stream as each q-chunk's AV completes."""
                qT, kT = st["qkT"]
                vts = st["vts"]
                it = 0
                ozf = [oz_p.tile([D + 1, tok], F32, tag="ozf", name=f"ozf{b}_{h}")
                       for h in range(HPC)]
                st["ozf"] = ozf
                for qc in range(n_qc):
                    q_sl = slice(qc * qc_w, (qc + 1) * qc_w)
                    ets = []
                    pavs = [ps_av.tile([D + 1, qc_w], F32, tag="av", name=f"pav{h}")
                            for h in range(HPC)]

                    def emit_sc(s):
                        psc = ps_sc.tile([128, 2 * qc_w], F32, tag="psc", name="psc")
                        for h in range(HPC):
                            nc.tensor.matmul(
                                psc[:, h * qc_w:(h + 1) * qc_w],
                                kT[h * D:(h + 1) * D, s * 128:(s + 1) * 128],
                                qT[h * D:(h + 1) * D, q_sl],
                                start=True, stop=True,
                            )
                        et = et_p.tile([128, 2 * qc_w], BF16, tag="et", name="et")
                        nc.scalar.activation(et[:], psc[:],
                                             mybir.ActivationFunctionType.Exp,
                                             scale=scale)
                        ets.append(et)

                    def emit_av(s):
                        for h in range(HPC):
                            nc.tensor.matmul(
                                pavs[h][:],
                                vts[s][:, h * (D + 1):(h + 1) * (D + 1)],
                                ets[s][:, h * qc_w:(h + 1) * qc_w],
                                start=(s == 0), stop=(s == n_s - 1),
                            )

                    for s in range(n_s):
                        emit_sc(s)
                        if s >= 2:
                            emit_av(s - 2)
                        if fills and not (b > 0 and it % 4 == 3):
                            fills.pop(0)()
                        it += 1
                    for s in range(max(0, n_s - 2), n_s):
                        emit_av(s)
                    for h in range(HPC):
                        nc.vector.tensor_copy(ozf[h][:, q_sl], pavs[h][:])
                    if self_push:
                        fills.extend(zproj_qc_quanta(b, st, qc))
                # drain any leftover fill quanta
                while fills:
                    fills.pop(0)()

            def interleave2(a, bq):
                out = []
                ia = ib = 0
                while ia < len(a) or ib < len(bq):
                    for _ in range(2):
                        if ia < len(a):
                            out.append(a[ia]); ia += 1
                    if ib < len(bq):
                        out.append(bq[ib]); ib += 1
                return out

            # HAM warm-up: ~60 throwaway matmuls on a zeroed tile run while
            # the first x tiles stream in, so the real QKV starts at 2.4 GHz
            # instead of paying the ~3.4us cold window at half clock.
            warm_sb = singles.tile([128, 128], BF16)
            nc.gpsimd.memset(warm_sb[:], 0.0)
            warm_ps = ps_mm.tile([128, 128], F32, tag="mm", name="warm_ps")
            for i in range(60):
                nc.tensor.matmul(warm_ps[:], warm_sb[:], warm_sb[:],
                                 start=(i == 0), stop=(i == 59))

            states = [dict() for _ in range(n_batch)]
            q0 = make_qkv_quanta(0, states[0])
            # batch 0: x loads + qk groups up front; v groups become fills
            n_up = 1 + 2 * n_qk
            for q in q0[:n_up]:
                q()
            carry = q0[n_up:]
            prj = []
            for b in range(n_batch):
                last = b == n_batch - 1
                fills = list(carry)
                carry = []
                nxt = make_qkv_quanta(b + 1, states[b + 1]) if not last else []
                if nxt:
                    fills.append(nxt.pop(0))  # x loads first
                fills += prj[:2]              # first z chains early
                fills += interleave2(nxt, prj[2:])
                phase_attn(b, states[b], fills, self_push=last)
                if not last:
                    prj = []
                    for qc in range(n_qc):
                        prj += zproj_qc_quanta(b, states[b], qc)

    nc.compile()
    return nc


def prep_in_maps(x, W_qkv, W_proj, n_batch=B, tok=N):
    """Shard + lay out inputs per core (bf16, transposed as the kernel wants)."""
    t_all = n_batch * tok
    x2 = np.ascontiguousarray(
        np.asarray(x, dtype=np.float32).reshape(t_all, C).T).astype(nbf16)
    Wq = np.asarray(W_qkv[0:C], dtype=np.float32)
    Wk = np.asarray(W_qkv[C:2 * C], dtype=np.float32)
    Wv = np.asarray(W_qkv[2 * C:3 * C], dtype=np.float32)
    Wp = np.asarray(W_proj, dtype=np.float32)
    in_maps = []
    for cid in range(NCORES):
        h0, h1 = HPC * cid, HPC * cid + 1
        r0, r1 = slice(h0 * D, (h0 + 1) * D), slice(h1 * D, (h1 + 1) * D)
        wqk = np.concatenate([Wq[r0], Wq[r1], Wk[r0], Wk[r1]], axis=0)
        wv = np.concatenate([Wv[r0], Wv[r1]], axis=0)
        wp = np.concatenate([Wp[:, r0], Wp[:, r1]], axis=1)
        in_maps.append({
            "xT": x2,
            "wqkT": np.ascontiguousarray(wqk.T).astype(nbf16),
            "wvT": np.ascontiguousarray(wv.T).astype(nbf16),
            "wpT": np.ascontiguousarray(wp.T).astype(nbf16),
        })
    return in_maps


_CACHE = {}


def run(x, W_qkv, W_proj, b_proj, trace=False, trace_kwargs=None):
    key = "full"
    if key not in _CACHE:
        _CACHE[key] = build()
    nc = _CACHE[key]
    in_maps = prep_in_maps(x, W_qkv, W_proj)
    res = run_bass_kernel_spmd(
        nc, in_maps, core_ids=list(range(NCORES)), trace=trace,
        **(trace_kwargs or {}))
    acc = res.results[0]["out"].astype(np.float32)
    for i in range(1, NCORES):
        acc += res.results[i]["out"]
    acc += np.asarray(b_proj, dtype=np.float32)[None, :]
    return acc.reshape(B, N, C), res


def kernel(x, W_qkv, W_proj, b_proj):
    out, _ = run(x, W_qkv, W_proj, b_proj)
    return out
